# revision 1
# baseline (speedup 1.0000x reference)
"""DGP loss kernel for Trainium2, 8 NeuronCores, pure data parallel.

Math (algebraically identical to the reference):
  - The reference extracts overlapping 5x5 patches (stride 1) of the 4x-downsampled
    depth grid and takes a masked lower-median over each 4x4 depth sub-block.
    Sub-block (u,v) of patch (a,b) is exactly the aligned 4x4 depth block at
    feature-grid cell (a+u, b+v), so we compute the median once per cell:
    M[i,j], i<128, j<256.
  - Since normalized disparity n(d) is monotone DECREASING in d, the lower
    median of the n-values equals n(idx-th LARGEST valid d), idx=(k-1)//2.
    We sort raw clamped depths descending (key = -d, invalid -> +BIG) and
    apply n() to the single selected value - no full-tensor reciprocal.
  - seg branch: with sf = F.normalize(seg_feat, dim=C), the patch term is
    ||sf_c - sf_n||^2 = 2 - 2*dot(sf_c, sf_n), so
    loss_term(center,offset) = exp(-|M_c - M_n|) * exp(2*dot - 2)
    summed over centers i in [2,126), j in [2,254) and the 24 offsets
    (du,dv) in [-2,2]^2 minus (0,0); valid_amount = sum (M_c>0)&(M_n>0).
  - per_img = sum(terms)/max(valid_amount,1); loss = mean over images.

Sharding: 8 cores = 2 images x 4 column bands (63 centers each, +2 halo).

Layouts (engine APs must start at partition 0/32/64/96, so row shifts must
live in the free dimension):
  - depth/median work: [i=128 partitions, free] (no shifted operands needed)
  - correlation: sf in strip layout [(s:4 x c:32)=128 partitions,
    (ii:35, j:68 padded) free]; strip s holds feature rows 31s..31s+35; both
    du and dv shifts are free-dim offsets. The row stride is padded to 68 so
    bf16 operand starts keep 4-byte alignment (dv odd handled by a 1-column
    pre-shifted bf16 copy). Channel reduction via PE matmul with a
    block-diagonal ones lhsT -> PSUM [4, 31*64]; an ACT copy (bf16) + DMA
    regroups (s, ii) -> center-row partitions.
  - shifted copies of M (5 row shifts) via SBUF->SBUF DMAs (DMA has no
    partition-start restriction).
"""

from contextlib import ExitStack

import numpy as np

import concourse.bass as bass
import concourse.mybir as mybir
import concourse.tile as tile
from concourse import bass_utils
from concourse.alu_op_type import AluOpType

F32 = mybir.dt.float32
BF16 = mybir.dt.bfloat16
AF = mybir.ActivationFunctionType

EPS_FN = 1e-8
BIG = 3.0e38
MIN_D, MAX_D = 1.0, 100.0
NSCALE = 1.0 / (1.0 / MIN_D - 1.0 / MAX_D)          # 1/0.99
NBIAS = -(1.0 / MAX_D) * NSCALE                      # -0.01/0.99
# n(d) > EPS_FN  <=>  d < DD_THR (n is monotone decreasing)
DD_THR = float(1.0 / (1.0 / MAX_D + EPS_FN / NSCALE))

# Per-core shard geometry (2 images x 4 bands of 63 centers).
JBAND = 63          # centers per band
JCOLS = 67          # feature cols loaded (halo 2 each side)
JP = 68             # padded row stride (even, for bf16 4B alignment)
JW = 64             # padded correlation width (63 centers + 1 junk col)
DCOLS = 4 * JCOLS   # depth cols loaded
NROW = 35           # feature rows per strip (31 centers + 4 halo)
NCEN = 31           # center rows per strip
NP = NCEN * JW      # per-offset dot count (incl junk col) = 1984
NF = NROW * JP      # strip free size = 2380


def _oddeven_merge_sort_layers(n):
    """Batcher odd-even mergesort compare-exchange pairs, grouped by layer."""
    layers = []
    p = 1
    while p < n:
        k = p
        while k >= 1:
            layer = []
            for j in range(k % p, n - k, 2 * k):
                for i in range(0, min(k, n - j - k)):
                    if (i + j) // (p * 2) == (i + j + k) // (p * 2):
                        layer.append((i + j, i + j + k))
            layers.append(layer)
            k //= 2
        p *= 2
    return layers


def _group_runs(pairs):
    """Group CE pairs (a, a+d) of one layer into (a0, step, count, d) runs
    with arithmetic-progression a's so each run is one strided AP op."""
    by_d = {}
    for a, b in pairs:
        by_d.setdefault(b - a, []).append(a)
    runs = []
    for d, alist in sorted(by_d.items()):
        alist = sorted(alist)
        i = 0
        while i < len(alist):
            j = i + 1
            step = None
            while j < len(alist):
                s = alist[j] - alist[j - 1]
                if step is None:
                    step = s
                elif s != step:
                    break
                j += 1
            cnt = j - i
            runs.append((alist[i], step if cnt > 1 else 1, cnt, d))
            i = j
    return runs


def _planes(t, start, step, count):
    """AP over plane dim of a [128, NPLANES, W] tile."""
    if count == 1:
        return t[:, start : start + 1, :]
    return t[:, start : start + (count - 1) * step + 1 : step, :]


def _split_excess_waits(nc, max_waits=1):
    """This container's walrus build rejects instructions carrying more than
    one sem-wait ("Too many sync wait commands"); Tile's scheduler happily
    attaches several. Move the excess onto standalone EventSemaphore waits
    immediately before the instruction on the same engine queue."""
    for f in nc.m.functions:
        for blk in f.blocks:
            new_insts = []
            for inst in blk.instructions:
                si = inst.sync_info
                if si is not None and si.on_wait and len(si.on_wait) > max_waits:
                    waits = list(si.on_wait)
                    excess, keep = waits[:-max_waits], waits[-max_waits:]
                    idx = 0
                    while excess:
                        chunk, excess = excess[:max_waits], excess[max_waits:]
                        new_insts.append(
                            mybir.InstEventSemaphore(
                                name=f"{inst.name}-wsplit{idx}",
                                engine=inst.engine,
                                ins=[],
                                outs=[],
                                sync_info=mybir.SyncInfo(on_wait=chunk, on_update=[]),
                            )
                        )
                        idx += 1
                    si.on_wait = keep
                new_insts.append(inst)
            blk.instructions[:] = new_insts


def _act_rsqrt(nc, out, in_, bias_ap):
    """Raw Rsqrt InstActivation: out = 1/sqrt(in_ + bias). bass's activation()
    refuses Rsqrt on accuracy grounds; our tolerance budget absorbs it and the
    end-to-end rel-err check guards the result."""
    act = nc.scalar
    inputs = [
        act.lower_ap(in_),
        act.lower_ap(bias_ap),
        mybir.ImmediateValue(dtype=mybir.dt.float32, value=1.0),
        mybir.ImmediateValue(dtype=mybir.dt.float32, value=0.0),
    ]
    return act.add_instruction(
        mybir.InstActivation(
            name=nc.get_next_instruction_name(),
            func=AF.Rsqrt,
            ins=inputs,
            outs=[act.lower_ap(out)],
        )
    )


def _build_core_program(split_waits=True):
    nc = bass.Bass("TRN2", target_bir_lowering=False, debug=False)
    dep = nc.dram_tensor("dep", [512, DCOLS], F32, kind="ExternalInput")
    sf = nc.dram_tensor("sf", [32, 128, JCOLS], F32, kind="ExternalInput")
    out = nc.dram_tensor("out", [124, 4], F32, kind="ExternalOutput")

    with tile.TileContext(nc) as tc, ExitStack() as ctx:
        persist = ctx.enter_context(tc.tile_pool(name="persist", bufs=1))
        work = ctx.enter_context(tc.tile_pool(name="work", bufs=1))
        prods = ctx.enter_context(tc.tile_pool(name="prods", bufs=3))

        v = nc.vector
        act = nc.scalar

        # ---------------- depth branch ([i=128, ...] layout) ----------------
        dep_raw = work.tile([128, 4, DCOLS], F32)
        nc.sync.dma_start(
            out=dep_raw, in_=dep.ap().rearrange("(i r) w -> i r w", r=4)
        )
        dcl = work.tile([128, 4, DCOLS], F32)
        v.tensor_scalar(dcl, dep_raw, MIN_D, MAX_D, op0=AluOpType.max, op1=AluOpType.min)
        vld = work.tile([128, 4, DCOLS], F32)
        v.tensor_tensor(vld, dcl, dep_raw, op=AluOpType.is_equal)
        thrm = work.tile([128, 4, DCOLS], F32)
        v.tensor_scalar(thrm, dcl, DD_THR, None, op0=AluOpType.is_lt)
        valid = work.tile([128, 4, DCOLS], F32)
        v.tensor_tensor(valid, vld, thrm, op=AluOpType.mult)

        # valid count k per 4x4 block
        k_t = persist.tile([128, JCOLS], F32)
        v.reduce_sum(
            out=k_t,
            in_=valid.rearrange("p r (j s) -> p j r s", s=4),
            axis=mybir.AxisListType.XY,
        )

        # sort key: valid ? -d : +BIG  (ascending sort = descending depth);
        # built as (-1)*(valid*d) + BIG*(1-valid) to avoid f32 absorption
        dv_t = work.tile([128, 4, DCOLS], F32)
        v.tensor_tensor(dv_t, valid, dcl, op=AluOpType.mult)
        nvb = work.tile([128, 4, DCOLS], F32)
        v.tensor_scalar(nvb, valid, 0.5, BIG, op0=AluOpType.is_lt, op1=AluOpType.mult)
        ndv = work.tile([128, 4, DCOLS], F32)
        v.tensor_scalar(ndv, dv_t, -1.0, None, op0=AluOpType.mult)
        S = persist.tile([128, 16, JP], BF16)
        v.memset(S, BIG)
        v.tensor_tensor(
            out=S[:, :, 0:JCOLS].rearrange("p (r s) j -> p r j s", s=4),
            in0=ndv.rearrange("p r (j s) -> p r j s", s=4),
            in1=nvb.rearrange("p r (j s) -> p r j s", s=4),
            op=AluOpType.add,
        )
        for layer in _oddeven_merge_sort_layers(16):
            for a0, astep, cnt, d in _group_runs(layer):
                lo = _planes(S, a0, astep, cnt)
                hi = _planes(S, a0 + d, astep, cnt)
                tmp = prods.tile([128, cnt, JP], BF16, tag="cetmp")
                v.tensor_tensor(tmp[:, :cnt, :], lo, hi, op=AluOpType.max)
                v.tensor_tensor(lo, lo, hi, op=AluOpType.min)
                v.tensor_copy(hi, tmp[:, :cnt, :])

        # lower-median select: u_m = [k>=2m+1] - [k>=2m+3], m=0..7
        G = work.tile([128, 9, JCOLS], BF16)
        for m in range(9):
            v.tensor_scalar(G[:, m, :], k_t, float(2 * m + 1), None, op0=AluOpType.is_ge)
        u = work.tile([128, 8, JCOLS], BF16)
        v.tensor_tensor(u, G[:, 0:8, :], G[:, 1:9, :], op=AluOpType.subtract)
        sel = work.tile([128, 8, JCOLS], BF16)
        v.tensor_tensor(sel, S[:, 0:8, 0:JCOLS], u, op=AluOpType.mult)
        mdneg = work.tile([128, JCOLS], F32)
        v.reduce_sum(
            out=mdneg, in_=sel.rearrange("p m j -> p j m"), axis=mybir.AxisListType.X
        )
        # med_d = max(-mdneg, 1); M = (1/med_d * NSCALE + NBIAS) * [k>=1]
        med_d = work.tile([128, JCOLS], F32)
        v.tensor_scalar(med_d, mdneg, -1.0, 1.0, op0=AluOpType.mult, op1=AluOpType.max)
        rec_s = work.tile([128, JCOLS], F32)
        v.reciprocal(rec_s, med_d)
        aff_s = work.tile([128, JCOLS], F32)
        act.activation(aff_s, rec_s, AF.Copy, bias=NBIAS, scale=NSCALE)
        # Kpos = [k>=1] is EXACTLY the reference's (median>0) mask, since all
        # valid disparities exceed EPS_FN; keep it in exact f32 (no bf16).
        Kpos = persist.tile([128, JP], F32)
        v.memset(Kpos, 0.0)
        v.tensor_scalar(Kpos[:, 0:JCOLS], k_t, 0.5, None, op0=AluOpType.is_ge)
        M = persist.tile([128, JP], F32)
        v.memset(M, 0.0)
        v.tensor_tensor(M[:, 0:JCOLS], aff_s, Kpos[:, 0:JCOLS], op=AluOpType.mult)

        # shifted copies xx_sh[d][p, j] = xx[row p+d, col j]
        M_sh = []
        K_sh = []
        for d in range(5):
            mt = persist.tile([124, JP], F32, tag=f"M_sh{d}")
            nc.sync.dma_start(out=mt, in_=M[d : d + 124, :])
            M_sh.append(mt)
            kt2 = persist.tile([124, JP], F32, tag=f"K_sh{d}")
            nc.sync.dma_start(out=kt2, in_=Kpos[d : d + 124, :])
            K_sh.append(kt2)

        # ---------------- seg branch (strip layout) ----------------
        sf_strip = persist.tile([128, NROW, JP], F32)
        v.memset(sf_strip[:, :, JCOLS:JP], 0.0)
        for s in range(4):
            nc.sync.dma_start(
                out=sf_strip[32 * s : 32 * (s + 1), :, 0:JCOLS],
                in_=sf.ap()[:, NCEN * s : NCEN * s + NROW, :],
            )

        # block-diagonal ones (f32 for the f32 nrm2 reduce, bf16 for prods)
        ones4f = persist.tile([128, 4], F32)
        v.memset(ones4f, 0.0)
        ones4b = persist.tile([128, 4], BF16)
        v.memset(ones4b, 0.0)
        for s in range(4):
            v.memset(ones4f[32 * s : 32 * (s + 1), s : s + 1], 1.0)
            v.memset(ones4b[32 * s : 32 * (s + 1), s : s + 1], 1.0)
        eps_b = persist.tile([4, 1], F32)
        v.memset(eps_b, 1e-24)

        f2 = work.tile([128, NROW, JP], F32)
        v.tensor_tensor(f2, sf_strip, sf_strip, op=AluOpType.mult)
        rinv = work.tile([4, NF], F32)
        with tc.tile_pool(name="psnrm", bufs=1, space="PSUM") as psnrm:
            nrm2_ps = psnrm.tile([4, NF], F32, tag="nrm2")
            for c0 in range(0, NF, 512):
                cw = min(512, NF - c0)
                nc.tensor.matmul(
                    nrm2_ps[:, c0 : c0 + cw],
                    ones4f,
                    f2.rearrange("p a b -> p (a b)")[:, c0 : c0 + cw],
                )
            # rinv = 1/sqrt(nrm2 + 1e-24)  (pad cols are all-zero)
            _act_rsqrt(nc, rinv, nrm2_ps, eps_b)
        # broadcast rinv [4, NF] -> [128, NF] (strip row s -> partitions 32s..):
        # DMA with a step-0 free dim on the source (partition step-0 is
        # rejected by the DMA lowering, free-dim replication is fine)
        rinv_rep = work.tile([128, NF], F32)
        for s in range(4):
            nc.sync.dma_start(
                out=rinv_rep[32 * s : 32 * (s + 1), :],
                in_=rinv[s : s + 1, :].unsqueeze(1).broadcast_to((1, 32, NF)),
            )
        sfn = persist.tile([128, NROW, JP], F32)
        v.tensor_tensor(
            sfn.rearrange("p a b -> p (a b)"),
            sf_strip.rearrange("p a b -> p (a b)"),
            rinv_rep,
            op=AluOpType.mult,
        )
        sfb = persist.tile([128, NROW, JP], BF16)
        v.tensor_copy(sfb, sfn)
        # 1-col-left-shifted copy for odd dv offsets (keeps bf16 4B alignment)
        sfb1 = persist.tile([128, NROW, JP], BF16)
        v.memset(sfb1[:, :, JP - 1 : JP], 0.0)
        v.tensor_copy(sfb1[:, :, 0 : JP - 1], sfn[:, :, 1:JP])

        # ---------------- 25-offset correlation ----------------
        psum = ctx.enter_context(tc.tile_pool(name="psum", bufs=2, space="PSUM"))
        dots_b = persist.tile([124, 25, JW], BF16)
        dda = persist.tile([124, 25, JW], F32)
        vm = persist.tile([124, 25, JW], F32)

        cen = sfb[:, 2 : 2 + NCEN, 2 : 2 + JW]
        for o in range(25):
            du, dv = o // 5, o % 5
            src, dvv = (sfb1, dv - 1) if dv % 2 == 1 else (sfb, dv)
            par = src[:, du : du + NCEN, dvv : dvv + JW]
            prod = prods.tile([128, NCEN, JW], BF16, tag="prod")
            v.tensor_tensor(prod, cen, par, op=AluOpType.mult)
            dps = psum.tile([4, NP], F32, tag="dps")
            for c0 in range(0, NP, 512):
                cw = min(512, NP - c0)
                nc.tensor.matmul(
                    dps[:, c0 : c0 + cw],
                    ones4b,
                    prod.rearrange("p a b -> p (a b)")[:, c0 : c0 + cw],
                )
            # regroup PSUM [s, (ii, j)] -> dots[p = s*31+ii, o, j]
            dsb = prods.tile([4, NP], BF16, tag="dsb")
            act.activation(dsb, dps, AF.Copy)
            nc.sync.dma_start(
                out=dots_b[:, o, :],
                in_=dsb.rearrange("s (a b) -> s a b", a=NCEN),
            )

            Mc = M_sh[2][:, 2 : 2 + JW]
            Mn = M_sh[du][:, dv : dv + JW]
            v.tensor_tensor(dda[:, o, :], Mc, Mn, op=AluOpType.subtract)
            v.tensor_tensor(
                vm[:, o, :],
                K_sh[2][:, 2 : 2 + JW],
                K_sh[du][:, dv : dv + JW],
                op=AluOpType.mult,
            )

        act.activation(dda, dda, AF.Abs)

        dots_f = work.tile([124, 25, JW], F32)
        v.tensor_copy(dots_f, dots_b)
        xt = work.tile([124, 25, JW], F32)
        v.scalar_tensor_tensor(
            xt, dots_f, 2.0, dda, op0=AluOpType.mult, op1=AluOpType.subtract
        )
        bias_m2 = persist.tile([124, 1], F32)
        v.memset(bias_m2, -2.0)
        terms = work.tile([124, 25, JW], F32)
        act.activation(terms, xt, AF.Exp, bias=bias_m2, scale=1.0)

        numden = work.tile([124, 4], F32)
        v.reduce_sum(
            out=numden[:, 0:1], in_=terms[:, 0:12, 0:JBAND], axis=mybir.AxisListType.XY
        )
        v.reduce_sum(
            out=numden[:, 1:2], in_=terms[:, 13:25, 0:JBAND], axis=mybir.AxisListType.XY
        )
        v.reduce_sum(
            out=numden[:, 2:3], in_=vm[:, 0:12, 0:JBAND], axis=mybir.AxisListType.XY
        )
        v.reduce_sum(
            out=numden[:, 3:4], in_=vm[:, 13:25, 0:JBAND], axis=mybir.AxisListType.XY
        )
        nc.sync.dma_start(out=out.ap(), in_=numden)

    if split_waits:
        _split_excess_waits(nc)
    return nc


_NC_CACHE = []


def kernel(seg_feat: np.ndarray, dep_true: np.ndarray) -> np.ndarray:
    seg_feat = np.ascontiguousarray(seg_feat, dtype=np.float32)
    dep_true = np.ascontiguousarray(dep_true, dtype=np.float32)

    if not _NC_CACHE:
        _NC_CACHE.append(_build_core_program())
    nc = _NC_CACHE[0]

    in_maps = []
    for core in range(8):
        img, band = core // 4, core % 4
        j0 = JBAND * band
        in_maps.append(
            {
                "dep": np.ascontiguousarray(dep_true[img, :, 4 * j0 : 4 * j0 + DCOLS]),
                "sf": np.ascontiguousarray(seg_feat[img, :, :, j0 : j0 + JCOLS]),
            }
        )

    res = bass_utils.run_bass_kernel_spmd(nc, in_maps, core_ids=list(range(8)))
    parts = [r["out"].astype(np.float64) for r in res.results]

    loss = 0.0
    for img in range(2):
        num = sum(parts[img * 4 + b][:, 0:2].sum() for b in range(4))
        den = sum(parts[img * 4 + b][:, 2:4].sum() for b in range(4))
        loss += num / max(den, 1.0)
    return np.float32(loss / 2.0)



# revision 8
# speedup vs baseline: 1.0464x; 1.0464x over previous
"""DGP loss kernel for Trainium2, 8 NeuronCores, pure data parallel.

Math (algebraically identical to the reference):
  - The reference extracts overlapping 5x5 patches (stride 1) of the 4x-downsampled
    depth grid and takes a masked lower-median over each 4x4 depth sub-block.
    Sub-block (u,v) of patch (a,b) is exactly the aligned 4x4 depth block at
    feature-grid cell (a+u, b+v), so we compute the median once per cell:
    M[i,j], i<128, j<256.
  - Since normalized disparity n(d) is monotone DECREASING in d, the lower
    median of the n-values equals n(idx-th LARGEST valid d), idx=(k-1)//2.
    We sort raw clamped depths descending (key = -d, invalid -> +BIG) and
    apply n() to the single selected value - no full-tensor reciprocal.
  - seg branch: with sf = F.normalize(seg_feat, dim=C), the patch term is
    ||sf_c - sf_n||^2 = 2 - 2*dot(sf_c, sf_n), so
    loss_term(center,offset) = exp(-|M_c - M_n|) * exp(2*dot - 2)
    summed over centers i in [2,126), j in [2,254) and the 24 offsets
    (du,dv) in [-2,2]^2 minus (0,0); valid_amount = sum (M_c>0)&(M_n>0).
  - per_img = sum(terms)/max(valid_amount,1); loss = mean over images.

Sharding: 8 cores = 2 images x 4 column bands (63 centers each, +2 halo).

Layouts (engine APs must start at partition 0/32/64/96, so row shifts must
live in the free dimension):
  - depth/median work: [i=128 partitions, free] (no shifted operands needed)
  - correlation: sf in strip layout [(s:4 x c:32)=128 partitions,
    (ii:35, j:68 padded) free]; strip s holds feature rows 31s..31s+35; both
    du and dv shifts are free-dim offsets. The row stride is padded to 68 so
    bf16 operand starts keep 4-byte alignment (dv odd handled by a 1-column
    pre-shifted bf16 copy). Channel reduction via PE matmul with a
    block-diagonal ones lhsT -> PSUM [4, 31*64]; an ACT copy (bf16) + DMA
    regroups (s, ii) -> center-row partitions.
  - shifted copies of M (5 row shifts) via SBUF->SBUF DMAs (DMA has no
    partition-start restriction).
"""

from contextlib import ExitStack

import numpy as np

import concourse.bass as bass
import concourse.mybir as mybir
import concourse.tile as tile
from concourse import bass_utils
from concourse.alu_op_type import AluOpType

F32 = mybir.dt.float32
BF16 = mybir.dt.bfloat16
AF = mybir.ActivationFunctionType

EPS_FN = 1e-8
BIG = 3.0e38
MIN_D, MAX_D = 1.0, 100.0
NSCALE = 1.0 / (1.0 / MIN_D - 1.0 / MAX_D)          # 1/0.99
NBIAS = -(1.0 / MAX_D) * NSCALE                      # -0.01/0.99
# n(d) > EPS_FN  <=>  d < DD_THR (n is monotone decreasing)
DD_THR = float(1.0 / (1.0 / MAX_D + EPS_FN / NSCALE))

# Per-core shard geometry (2 images x 4 bands of 63 centers).
JBAND = 63          # centers per band
JCOLS = 67          # feature cols loaded (halo 2 each side)
JP = 68             # padded row stride (even, for bf16 4B alignment)
JW = 64             # padded correlation width (63 centers + 1 junk col)
DCOLS = 4 * JCOLS   # depth cols loaded
NROW = 35           # feature rows per strip (31 centers + 4 halo)
NCEN = 31           # center rows per strip
NP = NCEN * JW      # per-offset dot count (incl junk col) = 1984
NF = NROW * JP      # strip free size = 2380


# Batcher odd-even mergesort on 16 planes, with every layer's lo/hi pair set
# expressed as ONE rectangular AP view (Q = split of the plane index n = g*Q+q,
# then [gslice, qslice]); 10 layers x 3 vector ops total.
# Each entry: (Q, lo_g, lo_q, hi_g, hi_q) with slices in the (g, q) grid.
_SORT_LAYERS = [
    (2, slice(0, 8), slice(0, 1), slice(0, 8), slice(1, 2)),        # p1k1
    (4, slice(0, 4), slice(0, 2), slice(0, 4), slice(2, 4)),        # p2k2
    (4, slice(0, 4), slice(1, 2), slice(0, 4), slice(2, 3)),        # p2k1
    (8, slice(0, 2), slice(0, 4), slice(0, 2), slice(4, 8)),        # p4k4
    (8, slice(0, 2), slice(2, 4), slice(0, 2), slice(4, 6)),        # p4k2
    (8, slice(0, 2), slice(1, 6, 2), slice(0, 2), slice(2, 7, 2)),  # p4k1
    (16, slice(0, 1), slice(0, 8), slice(0, 1), slice(8, 16)),      # p8k8
    (16, slice(0, 1), slice(4, 8), slice(0, 1), slice(8, 12)),      # p8k4
    (4, slice(0, 3), slice(2, 4), slice(1, 4), slice(0, 2)),        # p8k2
    (2, slice(0, 7), slice(1, 2), slice(1, 8), slice(0, 1)),        # p8k1
]


def _sort_views(S, Q, gs, qs):
    """AP over planes of S [128, 16, W] on the (g, q) grid with n = g*Q + q."""
    v = S.rearrange("p (g q) c -> p g q c", q=Q)
    return v[:, gs, qs, :]


def _split_excess_waits(nc, max_waits=1):
    """This container's walrus build rejects instructions carrying more than
    one sem-wait ("Too many sync wait commands"); Tile's scheduler happily
    attaches several. Move the excess onto standalone EventSemaphore waits
    immediately before the instruction on the same engine queue."""
    for f in nc.m.functions:
        for blk in f.blocks:
            new_insts = []
            for inst in blk.instructions:
                si = inst.sync_info
                if si is not None and si.on_wait and len(si.on_wait) > max_waits:
                    waits = list(si.on_wait)
                    excess, keep = waits[:-max_waits], waits[-max_waits:]
                    idx = 0
                    while excess:
                        chunk, excess = excess[:max_waits], excess[max_waits:]
                        new_insts.append(
                            mybir.InstEventSemaphore(
                                name=f"{inst.name}-wsplit{idx}",
                                engine=inst.engine,
                                ins=[],
                                outs=[],
                                sync_info=mybir.SyncInfo(on_wait=chunk, on_update=[]),
                            )
                        )
                        idx += 1
                    si.on_wait = keep
                new_insts.append(inst)
            blk.instructions[:] = new_insts


def _act_rsqrt(nc, out, in_, bias_ap):
    """Raw Rsqrt InstActivation: out = 1/sqrt(in_ + bias). bass's activation()
    refuses Rsqrt on accuracy grounds; our tolerance budget absorbs it and the
    end-to-end rel-err check guards the result."""
    act = nc.scalar
    inputs = [
        act.lower_ap(in_),
        act.lower_ap(bias_ap),
        mybir.ImmediateValue(dtype=mybir.dt.float32, value=1.0),
        mybir.ImmediateValue(dtype=mybir.dt.float32, value=0.0),
    ]
    return act.add_instruction(
        mybir.InstActivation(
            name=nc.get_next_instruction_name(),
            func=AF.Rsqrt,
            ins=inputs,
            outs=[act.lower_ap(out)],
        )
    )


def _build_core_program(split_waits=True):
    nc = bass.Bass("TRN2", target_bir_lowering=False, debug=False)
    dep = nc.dram_tensor("dep", [512, DCOLS], F32, kind="ExternalInput")
    sf = nc.dram_tensor("sf", [32, 128, JP], F32, kind="ExternalInput")
    out = nc.dram_tensor("out", [124, 4], F32, kind="ExternalOutput")

    with tile.TileContext(nc) as tc, ExitStack() as ctx:
        persist = ctx.enter_context(tc.tile_pool(name="persist", bufs=1))
        work = ctx.enter_context(tc.tile_pool(name="work", bufs=1))
        prods = ctx.enter_context(tc.tile_pool(name="prods", bufs=3))

        v = nc.vector
        act = nc.scalar

        # ---------------- depth branch ([i=128, ...] layout) ----------------
        dep_raw = work.tile([128, 4, DCOLS], F32)
        nc.sync.dma_start(
            out=dep_raw, in_=dep.ap().rearrange("(i r) w -> i r w", r=4)
        )
        dcl = work.tile([128, 4, DCOLS], F32)
        v.tensor_scalar(dcl, dep_raw, MIN_D, MAX_D, op0=AluOpType.max, op1=AluOpType.min)
        vld = work.tile([128, 4, DCOLS], F32)
        v.tensor_tensor(vld, dcl, dep_raw, op=AluOpType.is_equal)
        thrm = work.tile([128, 4, DCOLS], F32)
        v.tensor_scalar(thrm, dcl, DD_THR, None, op0=AluOpType.is_lt)
        valid = work.tile([128, 4, DCOLS], F32)
        v.tensor_tensor(valid, vld, thrm, op=AluOpType.mult)

        # valid count k per 4x4 block
        k_t = persist.tile([128, JCOLS], F32)
        v.reduce_sum(
            out=k_t,
            in_=valid.rearrange("p r (j s) -> p j r s", s=4),
            axis=mybir.AxisListType.XY,
        )

        # sort key: valid ? -d : +BIG  (ascending sort = descending depth);
        # built as (-1)*(valid*d) + BIG*(1-valid) to avoid f32 absorption
        dv_t = work.tile([128, 4, DCOLS], F32)
        v.tensor_tensor(dv_t, valid, dcl, op=AluOpType.mult)
        nvb = work.tile([128, 4, DCOLS], F32)
        v.tensor_scalar(nvb, valid, 0.5, BIG, op0=AluOpType.is_lt, op1=AluOpType.mult)
        ndv = work.tile([128, 4, DCOLS], F32)
        v.tensor_scalar(ndv, dv_t, -1.0, None, op0=AluOpType.mult)
        S = persist.tile([128, 16, JP], BF16)
        v.memset(S, BIG)
        v.tensor_tensor(
            out=S[:, :, 0:JCOLS].rearrange("p (r s) j -> p r j s", s=4),
            in0=ndv.rearrange("p r (j s) -> p r j s", s=4),
            in1=nvb.rearrange("p r (j s) -> p r j s", s=4),
            op=AluOpType.add,
        )
        for Q, lg, lq, hg, hq in _SORT_LAYERS:
            lo = _sort_views(S, Q, lg, lq)
            hi = _sort_views(S, Q, hg, hq)
            npl = lo.shape[1] * lo.shape[2]
            tmp = prods.tile([128, npl, JP], BF16, tag="cetmp")
            tv = tmp.rearrange("p (g q) c -> p g q c", q=lo.shape[2])
            v.tensor_tensor(tv, lo, hi, op=AluOpType.max)
            v.tensor_tensor(lo, lo, hi, op=AluOpType.min)
            v.tensor_copy(hi, tv)

        # lower-median select: u_m = [k>=2m+1] - [k>=2m+3], m=0..7
        G = work.tile([128, 9, JCOLS], BF16)
        for m in range(9):
            v.tensor_scalar(G[:, m, :], k_t, float(2 * m + 1), None, op0=AluOpType.is_ge)
        u = work.tile([128, 8, JCOLS], BF16)
        v.tensor_tensor(u, G[:, 0:8, :], G[:, 1:9, :], op=AluOpType.subtract)
        sel = work.tile([128, 8, JCOLS], BF16)
        v.tensor_tensor(sel, S[:, 0:8, 0:JCOLS], u, op=AluOpType.mult)
        mdneg = work.tile([128, JCOLS], F32)
        v.reduce_sum(
            out=mdneg, in_=sel.rearrange("p m j -> p j m"), axis=mybir.AxisListType.X
        )
        # med_d = max(-mdneg, 1); M = (1/med_d * NSCALE + NBIAS) * [k>=1]
        med_d = work.tile([128, JCOLS], F32)
        v.tensor_scalar(med_d, mdneg, -1.0, 1.0, op0=AluOpType.mult, op1=AluOpType.max)
        rec_s = work.tile([128, JCOLS], F32)
        v.reciprocal(rec_s, med_d)
        aff_s = work.tile([128, JCOLS], F32)
        act.activation(aff_s, rec_s, AF.Copy, bias=NBIAS, scale=NSCALE)
        # Kpos = [k>=1] is EXACTLY the reference's (median>0) mask, since all
        # valid disparities exceed EPS_FN; keep it in exact f32 (no bf16).
        Kpos = persist.tile([128, JP], F32)
        v.memset(Kpos, 0.0)
        v.tensor_scalar(Kpos[:, 0:JCOLS], k_t, 0.5, None, op0=AluOpType.is_ge)
        M = persist.tile([128, JP], F32)
        v.memset(M, 0.0)
        v.tensor_tensor(M[:, 0:JCOLS], aff_s, Kpos[:, 0:JCOLS], op=AluOpType.mult)

        # shifted copies MS[p, d, j] = M[row p+d, col j] (one stacked tile)
        MS = persist.tile([124, 5, JP], F32)
        for d in range(5):
            nc.sync.dma_start(out=MS[:, d, :], in_=M[d : d + 124, :])

        # ---------------- seg branch (strip layout) ----------------
        # sf arrives host-padded to JP cols, so each strip loads as one
        # contiguous 35*68*4B packet per channel partition.
        sf_strip = persist.tile([128, NROW, JP], F32)
        for s in range(4):
            nc.sync.dma_start(
                out=sf_strip[32 * s : 32 * (s + 1), :, :],
                in_=sf.ap()[:, NCEN * s : NCEN * s + NROW, :],
            )

        # block-diagonal ones (f32 for the f32 nrm2 reduce, bf16 for prods)
        ones4f = persist.tile([128, 4], F32)
        v.memset(ones4f, 0.0)
        ones4b = persist.tile([128, 4], BF16)
        v.memset(ones4b, 0.0)
        for s in range(4):
            v.memset(ones4f[32 * s : 32 * (s + 1), s : s + 1], 1.0)
            v.memset(ones4b[32 * s : 32 * (s + 1), s : s + 1], 1.0)
        eps_b = persist.tile([4, 1], F32)
        v.memset(eps_b, 1e-24)

        f2 = work.tile([128, NROW, JP], F32)
        v.tensor_tensor(f2, sf_strip, sf_strip, op=AluOpType.mult)
        rinv = work.tile([4, NF], F32)
        with tc.tile_pool(name="psnrm", bufs=1, space="PSUM") as psnrm:
            nrm2_ps = psnrm.tile([4, NF], F32, tag="nrm2")
            for c0 in range(0, NF, 512):
                cw = min(512, NF - c0)
                nc.tensor.matmul(
                    nrm2_ps[:, c0 : c0 + cw],
                    ones4f,
                    f2.rearrange("p a b -> p (a b)")[:, c0 : c0 + cw],
                )
            # rinv = 1/sqrt(nrm2 + 1e-24)  (pad cols are all-zero)
            _act_rsqrt(nc, rinv, nrm2_ps, eps_b)
        # broadcast rinv [4, NF] -> [128, NF] (strip row s -> partitions 32s..):
        # DMA with a step-0 free dim on the source (partition step-0 is
        # rejected by the DMA lowering, free-dim replication is fine)
        rinv_rep = work.tile([128, NF], F32)
        for s in range(4):
            nc.sync.dma_start(
                out=rinv_rep[32 * s : 32 * (s + 1), :],
                in_=rinv[s : s + 1, :].unsqueeze(1).broadcast_to((1, 32, NF)),
            )
        sfn = persist.tile([128, NROW, JP], F32)
        v.tensor_tensor(
            sfn.rearrange("p a b -> p (a b)"),
            sf_strip.rearrange("p a b -> p (a b)"),
            rinv_rep,
            op=AluOpType.mult,
        )
        sfb = persist.tile([128, NROW, JP], BF16)
        v.tensor_copy(sfb, sfn)
        # 1-col-left-shifted copy for odd dv offsets (keeps bf16 4B alignment)
        sfb1 = persist.tile([128, NROW, JP], BF16)
        v.memset(sfb1[:, :, JP - 1 : JP], 0.0)
        v.tensor_copy(sfb1[:, :, 0 : JP - 1], sfn[:, :, 1:JP])

        # ---------------- 25-offset correlation ----------------
        psum = ctx.enter_context(tc.tile_pool(name="psum", bufs=2, space="PSUM"))
        dots_b = persist.tile([124, 25, JW], BF16)
        dda = persist.tile([124, 25, JW], F32)
        vm = persist.tile([124, 25, JW], F32)

        cen = sfb[:, 2 : 2 + NCEN, 2 : 2 + JW]
        for o in range(25):
            du, dv = o // 5, o % 5
            src, dvv = (sfb1, dv - 1) if dv % 2 == 1 else (sfb, dv)
            par = src[:, du : du + NCEN, dvv : dvv + JW]
            prod = prods.tile([128, NCEN, JW], BF16, tag="prod")
            v.tensor_tensor(prod, cen, par, op=AluOpType.mult)
            dps = psum.tile([4, NP], F32, tag="dps")
            for c0 in range(0, NP, 512):
                cw = min(512, NP - c0)
                nc.tensor.matmul(
                    dps[:, c0 : c0 + cw],
                    ones4b,
                    prod.rearrange("p a b -> p (a b)")[:, c0 : c0 + cw],
                )
            # regroup PSUM [s, (ii, j)] -> dots[p = s*31+ii, o, j]
            dsb = prods.tile([4, NP], BF16, tag="dsb")
            act.activation(dsb, dps, AF.Copy)
            nc.sync.dma_start(
                out=dots_b[:, o, :],
                in_=dsb.rearrange("s (a b) -> s a b", a=NCEN),
            )

        # dda/vm batched by dv: o = 5*du + dv, so fixed dv -> stride-5 o-planes.
        # vm = [Kc*Kn] == [min(Mc, Mn) > 0] since M > 0 exactly where Kpos = 1.
        Mc_b = MS[:, 2, 2 : 2 + JW].unsqueeze(1).broadcast_to((124, 5, JW))
        for dv in range(5):
            Mn_g = MS[:, :, dv : dv + JW]
            v.tensor_tensor(
                dda[:, dv : dv + 21 : 5, :], Mc_b, Mn_g, op=AluOpType.subtract
            )
            v.tensor_tensor(
                vm[:, dv : dv + 21 : 5, :], Mc_b, Mn_g, op=AluOpType.min
            )
        v.tensor_scalar(vm, vm, 0.0, None, op0=AluOpType.is_gt)
        act.activation(dda, dda, AF.Abs)

        dots_f = work.tile([124, 25, JW], F32)
        v.tensor_copy(dots_f, dots_b)
        xt = work.tile([124, 25, JW], F32)
        v.scalar_tensor_tensor(
            xt, dots_f, 2.0, dda, op0=AluOpType.mult, op1=AluOpType.subtract
        )
        bias_m2 = persist.tile([124, 1], F32)
        v.memset(bias_m2, -2.0)
        terms = work.tile([124, 25, JW], F32)
        act.activation(terms, xt, AF.Exp, bias=bias_m2, scale=1.0)

        numden = work.tile([124, 4], F32)
        v.reduce_sum(
            out=numden[:, 0:1], in_=terms[:, 0:12, 0:JBAND], axis=mybir.AxisListType.XY
        )
        v.reduce_sum(
            out=numden[:, 1:2], in_=terms[:, 13:25, 0:JBAND], axis=mybir.AxisListType.XY
        )
        v.reduce_sum(
            out=numden[:, 2:3], in_=vm[:, 0:12, 0:JBAND], axis=mybir.AxisListType.XY
        )
        v.reduce_sum(
            out=numden[:, 3:4], in_=vm[:, 13:25, 0:JBAND], axis=mybir.AxisListType.XY
        )
        nc.sync.dma_start(out=out.ap(), in_=numden)

    if split_waits:
        _split_excess_waits(nc)
    return nc


_NC_CACHE = []


def kernel(seg_feat: np.ndarray, dep_true: np.ndarray) -> np.ndarray:
    seg_feat = np.ascontiguousarray(seg_feat, dtype=np.float32)
    dep_true = np.ascontiguousarray(dep_true, dtype=np.float32)

    if not _NC_CACHE:
        _NC_CACHE.append(_build_core_program())
    nc = _NC_CACHE[0]

    in_maps = []
    for core in range(8):
        img, band = core // 4, core % 4
        j0 = JBAND * band
        sfp = np.zeros((32, 128, JP), dtype=np.float32)
        sfp[:, :, 0:JCOLS] = seg_feat[img, :, :, j0 : j0 + JCOLS]
        in_maps.append(
            {
                "dep": np.ascontiguousarray(dep_true[img, :, 4 * j0 : 4 * j0 + DCOLS]),
                "sf": sfp,
            }
        )

    res = bass_utils.run_bass_kernel_spmd(nc, in_maps, core_ids=list(range(8)))
    parts = [r["out"].astype(np.float64) for r in res.results]

    loss = 0.0
    for img in range(2):
        num = sum(parts[img * 4 + b][:, 0:2].sum() for b in range(4))
        den = sum(parts[img * 4 + b][:, 2:4].sum() for b in range(4))
        loss += num / max(den, 1.0)
    return np.float32(loss / 2.0)

